# revision 1
# baseline (speedup 1.0000x reference)
import sys

sys.path.insert(0, "/opt/trn_rl_repo")

import numpy as np
import ml_dtypes

import concourse.bass as bass
import concourse.tile as tile
from concourse import bacc, mybir
from concourse.bass_utils import run_bass_kernel_spmd

# Problem constants (hardcoded per contract)
B, N, F = 8, 512, 16
D, PH, PW = 150, 26, 26
IMG = 128
HP = IMG + 2 * PH  # 180 padded canvas rows
WP = IMG + 2 * PW  # 180 padded canvas cols
CSTRIDE = 184  # canvas row stride in SBUF (padded)
HW = PH * PW  # 676
C = 64  # spline coefficients per voxel
GRP = 128  # groups per core
EPG = 4  # emitters per group
EW = 32  # partition rows per emitter (F=16 + 16 zero pad, 32-aligned)
K = EPG * C  # 256 contraction (block diagonal)
KC = K // 128  # 2 K-chunks

# scatter split: first G_GP groups go to GPSIMD (restricted to a low-y band
# so its private canvas stays small), the rest to DVE.
G_GP = 42
USE_GP = False
ROW0 = 14  # smallest y_idx given input coordinate ranges
VROWS = 166 - ROW0  # DVE canvas rows [14, 165]
GROWS_MAX = 100  # GPSIMD canvas max rows -> band y < ROW0 + GROWS_MAX - 26

_compiled = None
_compiled_grows = None


def _build_bass(grows):
    nc = bacc.Bacc()
    f32 = mybir.dt.float32
    bf16 = mybir.dt.bfloat16
    i32 = mybir.dt.int32

    lhsT_d = nc.declare_dram_parameter("lhsT", [GRP, K, 128], bf16, isOutput=False)
    rhs_d = nc.declare_dram_parameter("rhs", [GRP, K, HW], bf16, isOutput=False)
    offs_d = nc.declare_dram_parameter("offs", [1, N], i32, isOutput=False)
    out_d = nc.declare_dram_parameter("out", [F, IMG * IMG], f32, isOutput=True)

    with tile.TileContext(nc) as tc:
        with (
            tc.tile_pool(name="canvas", bufs=1) as canvas_pool,
            tc.tile_pool(name="weights", bufs=3) as w_pool,
            tc.tile_pool(name="slabs", bufs=3) as s_pool,
            tc.tile_pool(name="psum", bufs=3, space="PSUM") as p_pool,
            tc.tile_pool(name="small", bufs=1) as small_pool,
        ):
            # +1 scratch row: the flat ds() slice of a patch claims PH full
            # rows even though only PW cols of the last row are touched
            canvas_v = canvas_pool.tile([EW, (VROWS + 1) * CSTRIDE], f32, tag="cv")
            canvas_g = canvas_pool.tile([EW, (grows + 1) * CSTRIDE], f32, tag="cg")
            nc.vector.memset(canvas_v[:], 0.0)
            (nc.gpsimd if USE_GP else nc.vector).memset(canvas_g[:], 0.0)

            offs_t = small_pool.tile([1, N], i32)
            nc.sync.dma_start(offs_t[:], offs_d[:])
            preg_v = nc.vector.alloc_register64("offp_v")
            preg_g = nc.gpsimd.alloc_register64("offp_g")

            for g in range(GRP):
                lt = w_pool.tile([128, KC * 128], bf16, tag="lt")
                nc.sync.dma_start(
                    lt[:].rearrange("p (kc m) -> p kc m", kc=KC),
                    lhsT_d[g].rearrange("(kc k) m -> k kc m", k=128),
                )
                rt = s_pool.tile([128, KC * HW], bf16, tag="rt")
                nc.sync.dma_start(
                    rt[:].rearrange("p (kc n) -> p kc n", kc=KC),
                    rhs_d[g].rearrange("(kc k) n -> k kc n", k=128),
                )
                ps = p_pool.tile([128, HW], f32, tag="ps")
                for kc in range(KC):
                    for n0, n1 in ((0, 512), (512, HW)):
                        nc.tensor.matmul(
                            ps[:, n0:n1],
                            lhsT=lt[:, kc * 128 : (kc + 1) * 128],
                            rhs=rt[:, kc * HW + n0 : kc * HW + n1],
                            start=(kc == 0),
                            stop=(kc == KC - 1),
                        )
                sbp = s_pool.tile([128, HW], f32, tag="sbp")
                nc.scalar.copy(out=sbp[:], in_=ps[:])
                ps3 = sbp[:].rearrange("p (h w) -> p h w", h=PH, w=PW)
                if g < G_GP:
                    if USE_GP:
                        eng, preg, cnv, rows = nc.gpsimd, preg_g, canvas_g, grows
                    else:
                        eng, preg, cnv, rows = nc.vector, preg_v, canvas_g, grows
                else:
                    eng, preg, cnv, rows = nc.vector, preg_v, canvas_v, VROWS
                for i in range(EPG):
                    e = g * EPG + i
                    if i % 2 == 0:
                        eng.reg_load(preg, offs_t[0:1, e : e + 2])
                    off = eng.snap(
                        preg.lo if i % 2 == 0 else preg.hi,
                        donate=True,
                        min_val=0,
                        max_val=(rows - PH) * CSTRIDE + (WP - PW),
                    )
                    dst = (
                        cnv[:, bass.ds(off, PH * CSTRIDE)]
                        .rearrange("p (h w) -> p h w", h=PH)[:, :, 0:PW]
                    )
                    eng.tensor_tensor(
                        out=dst,
                        in0=dst,
                        in1=ps3[EW * i : EW * (i + 1)],
                        op=mybir.AluOpType.add,
                    )

            # merge GPSIMD canvas into DVE canvas (shared row range), crop
            nc.vector.tensor_tensor(
                out=canvas_v[0:F, 0 : grows * CSTRIDE],
                in0=canvas_v[0:F, 0 : grows * CSTRIDE],
                in1=canvas_g[0:F, 0 : grows * CSTRIDE],
                op=mybir.AluOpType.add,
            )
            canvas3 = canvas_v[:, 0 : VROWS * CSTRIDE].rearrange(
                "p (h w) -> p h w", h=VROWS, w=CSTRIDE
            )
            nc.sync.dma_start(
                out_d[:].rearrange("p (h w) -> p h w", h=IMG, w=IMG),
                canvas3[0:F, PH - ROW0 : PH - ROW0 + IMG, PW : PW + IMG],
            )
    if not nc.is_finalized():
        nc.finalize()
    return nc


def _host_prep(xyz, n_photons, coeffs, inv_voxel_size, psf_center):
    """Per-batch host prep: indices, series, photon-folded lhsT, gathered rhs,
    engine assignment (low-y band -> GPSIMD groups)."""
    u = xyz * inv_voxel_size  # (B,N,3)
    u = u.copy()
    u[..., :2] -= psf_center[:2]
    u[..., 2] += psf_center[2]
    u_floor = np.floor(u)
    frac = u - u_floor
    ui = u_floor.astype(np.int32)
    x_idx = ui[..., 0] + PW  # (B,N)
    y_idx = ui[..., 1] + PH
    z_idx = ui[..., 2]
    frac[..., :2] = 1.0 - frac[..., :2]

    # 64-term series: series[b,n,c], c = kz*16 + kx*4 + ky
    p = frac[..., None] ** np.arange(4, dtype=np.float32)  # (B,N,3,4)
    vx, vy, vz = p[..., 0, :], p[..., 1, :], p[..., 2, :]
    series = (
        vz[..., :, None, None] * vx[..., None, :, None] * vy[..., None, None, :]
    ).reshape(B, N, C)

    series16 = n_photons[..., None] * series[:, :, None, :]  # (B,N,F,C)

    # pick the smallest y-band whose emitter count is >= GPSIMD slots on
    # every core; the band emitters scatter on GPSIMD into a small canvas.
    gp_slots = G_GP * EPG
    grows = None
    for yb in range(ROW0 + 26, ROW0 + GROWS_MAX - 25):
        if int(np.min(np.sum(y_idx < yb, axis=1))) >= gp_slots:
            grows = yb + 26 - ROW0
            break
    if grows is None:
        raise ValueError("no y-band with enough emitters for GPSIMD groups")

    # per-core emitter permutation: first gp_slots band emitters, then rest
    perm = np.empty((B, N), dtype=np.int64)
    for b in range(B):
        band = np.nonzero(y_idx[b] < yb)[0]
        rest = np.nonzero(y_idx[b] >= yb)[0]
        sel = band[:gp_slots]
        over = band[gp_slots:]
        perm[b] = np.concatenate([sel, over, rest])

    bi = np.arange(B)[:, None]
    s16p = series16[bi, perm]  # (B,N,F,C)
    zp = z_idx[bi, perm]
    yp = y_idx[bi, perm]
    xp = x_idx[bi, perm]

    # lhsT[b,g,(slot,c),(i,f)] block diagonal; 32-col blocks per emitter
    lhsT = np.zeros((B, GRP, K, 128), dtype=np.float32)
    s16g = s16p.reshape(B, GRP, EPG, F, C)
    for i in range(EPG):
        lhsT[:, :, i * C : (i + 1) * C, i * EW : i * EW + F] = s16g[
            :, :, i
        ].transpose(0, 1, 3, 2)
    lhsT = lhsT.astype(ml_dtypes.bfloat16)

    coeffs_t = np.ascontiguousarray(
        coeffs.reshape(D, HW, C).transpose(0, 2, 1)
    ).astype(ml_dtypes.bfloat16)  # (D, C, HW)
    rhs = coeffs_t[zp.reshape(-1)].reshape(B, GRP, K, HW)

    offs = ((yp - ROW0) * CSTRIDE + xp).astype(np.int32)  # (B,N)
    return lhsT, rhs, offs, grows


def kernel(xyz, n_photons, coeffs, inv_voxel_size, psf_center, img_size):
    global _compiled, _compiled_grows
    xyz = np.asarray(xyz, dtype=np.float32)
    n_photons = np.asarray(n_photons, dtype=np.float32)
    coeffs = np.asarray(coeffs, dtype=np.float32)
    inv_voxel_size = np.asarray(inv_voxel_size, dtype=np.float32)
    psf_center = np.asarray(psf_center, dtype=np.float32)

    lhsT, rhs, offs, grows = _host_prep(
        xyz, n_photons, coeffs, inv_voxel_size, psf_center
    )

    if _compiled is None or _compiled_grows != grows:
        _compiled = _build_bass(grows)
        _compiled_grows = grows
    nc = _compiled

    in_maps = [
        {"lhsT": lhsT[b], "rhs": rhs[b], "offs": offs[b : b + 1]}
        for b in range(B)
    ]
    res = run_bass_kernel_spmd(nc, in_maps, core_ids=list(range(B)))
    out = np.stack(
        [res.results[b]["out"].reshape(F, IMG, IMG) for b in range(B)], axis=0
    )
    return out



# revision 20
# speedup vs baseline: 1.2132x; 1.2132x over previous
import sys

sys.path.insert(0, "/opt/trn_rl_repo")

import numpy as np
import ml_dtypes

import concourse.bass as bass
import concourse.tile as tile
from concourse import bacc, mybir
from concourse.bass_utils import run_bass_kernel_spmd

# Problem constants (hardcoded per contract)
B, N, F = 8, 512, 16
D, PH, PW = 150, 26, 26
IMG = 128
C = 64  # spline coefficients per voxel
HW = PH * PW  # 676
NG = 132  # groups of 4 emitters; each group contracts K=128 = 2 stacked slabs
SLOTS = 4  # emitters per group; each slot owns 32 partitions (16 real + pad)
SW = 32  # slot width in partitions (engine partition bases must be 32-aligned)
ROW0 = 14  # min y_idx / x_idx given input coordinate ranges
CSTRIDE = 152  # canvas row stride (rows/cols span [14, 165) -> 151 + spill)
CV_ELEMS = 152 * CSTRIDE  # canvas elems per partition (incl. ds-claim spill)
OFF_MAX = (139 - ROW0) * CSTRIDE + (139 - ROW0)  # largest scatter offset

# Engine map: slot 3 (base 96) always GPSIMD into canvas slice [96:128];
# slot 1 (base 32) GPSIMD into slice [32:64] for the first GP1_EXTRA groups;
# everything else DVE into slice [0:32] (cross-partition-base reads allowed
# in multiples of 32).
USE_GP_SCATTER = False
GP1_EXTRA = 70


def _gp_slot(g, i):
    if not USE_GP_SCATTER:
        return False
    return i == 3 or (i == 1 and g < GP1_EXTRA)


N_GP = sum(_gp_slot(g, i) for g in range(NG) for i in range(SLOTS))
N_DVE = NG * SLOTS - N_GP
_compiled = None


def _build_bass():
    nc = bacc.Bacc()
    f32 = mybir.dt.float32
    bf16 = mybir.dt.bfloat16
    i32 = mybir.dt.int32

    # packed per-group buffer: [:, 0:128] lhsT, [:, 128:804] rhs slabs
    grp_d = nc.declare_dram_parameter("grp", [NG, 128, 128 + HW], bf16, isOutput=False)
    offs_v_d = nc.declare_dram_parameter("offs_v", [1, N_DVE + 1], i32, isOutput=False)
    offs_g_d = nc.declare_dram_parameter("offs_g", [1, N_GP + 1], i32, isOutput=False)
    # three canvas slices DMA'd out separately; host sums them
    out_d = nc.declare_dram_parameter("out", [3, F, IMG * IMG], bf16, isOutput=True)

    with tile.TileContext(nc) as tc:
        with (
            tc.tile_pool(name="canvas", bufs=1) as canvas_pool,
            tc.tile_pool(name="weights", bufs=4) as w_pool,
            tc.tile_pool(name="slabs", bufs=4) as s_pool,
            tc.tile_pool(name="psum", bufs=3, space="PSUM") as p_pool,
            tc.tile_pool(name="small", bufs=1) as small_pool,
        ):
            # one canvas tile; 32-aligned slices per engine:
            # [0:32] DVE, [32:64] GP slot-1, [96:128] GP slot-3
            canvas = canvas_pool.tile([128, CV_ELEMS], bf16, tag="cv")
            nc.scalar.memzero(canvas[:])

            offs_v_t = small_pool.tile([1, N_DVE + 1], i32)
            offs_g_t = small_pool.tile([1, N_GP + 1], i32)
            nc.sync.dma_start(offs_v_t[:], offs_v_d[:])
            nc.sync.dma_start(offs_g_t[:], offs_g_d[:])
            preg_v = nc.vector.alloc_register64("offp_v")
            preg_g = nc.gpsimd.alloc_register64("offp_g")

            nv = ng = 0
            for g in range(NG):
                gt = s_pool.tile([128, 128 + HW], bf16, tag="gt")
                nc.sync.dma_start(gt[:], grp_d[g])
                lt = gt[:, 0:128]
                rt = gt[:, 128 : 128 + HW]
                ps = p_pool.tile([128, HW], f32, tag="ps")
                for n0, n1 in ((0, 512), (512, HW)):
                    nc.tensor.matmul(
                        ps[:, n0:n1],
                        lhsT=lt,
                        rhs=rt[:, n0:n1],
                        start=True,
                        stop=True,
                    )
                sbp = s_pool.tile([128, HW], bf16, tag="sbp")
                nc.scalar.copy(out=sbp[:], in_=ps[:])
                ps3 = sbp[:].rearrange("p (h w) -> p h w", h=PH, w=PW)
                for i in range(SLOTS):
                    if _gp_slot(g, i):
                        eng, preg, offs_t, k = nc.gpsimd, preg_g, offs_g_t, ng
                        cnv = canvas[SW * i : SW * (i + 1)]  # same-base op
                        ng += 1
                    else:
                        eng, preg, offs_t, k = nc.vector, preg_v, offs_v_t, nv
                        cnv = canvas[0:SW]
                        nv += 1
                    if k % 2 == 0:
                        eng.reg_load(preg, offs_t[0:1, k : k + 2])
                    off = eng.snap(
                        preg.lo if k % 2 == 0 else preg.hi,
                        donate=True,
                        min_val=0,
                        max_val=OFF_MAX,
                    )
                    dst = (
                        cnv[:, bass.ds(off, PH * CSTRIDE)]
                        .rearrange("p (h w) -> p h w", h=PH)[:, :, 0:PW]
                    )
                    eng.tensor_tensor(
                        out=dst,
                        in0=dst,
                        in1=ps3[SW * i : SW * (i + 1)],
                        op=mybir.AluOpType.add,
                    )

            # DMA each engine's canvas slice crop out; host sums the three.
            # crop: rows/cols [26, 154) -> canvas-local [12, 140)
            crop_off = 12 * CSTRIDE + 12
            s1 = crop_off + IMG * CSTRIDE
            for j, base in enumerate((0, SW, 3 * SW)):
                cb = canvas[base : base + F, crop_off:s1].rearrange(
                    "p (h w) -> p h w", w=CSTRIDE
                )[:, :, 0:IMG]
                nc.sync.dma_start(
                    out_d[j].rearrange("p (h w) -> p h w", h=IMG, w=IMG),
                    cb,
                )
    if not nc.is_finalized():
        nc.finalize()
    return nc


def _pack_bins(z_idx):
    """Pack 512 emitters into bins of SLOTS slots, each bin drawing from at
    most 2 distinct z-buckets.  Buckets are grouped into units (singletons or
    pairs whose remainders mod SLOTS sum to <= SLOTS); each unit is laid out
    sequentially so every bin touches at most 2 buckets.  Returns list of NG
    bins: (emitter_idx_list, half_list, zA, zB); short bins = dead slots."""
    import collections

    buckets = collections.defaultdict(list)
    for e, z in enumerate(z_idx):
        buckets[int(z)].append(e)
    items = list(buckets.items())
    units = [[it] for it in items if len(it[1]) % SLOTS == 0]
    rn = sorted(
        (it for it in items if len(it[1]) % SLOTS != 0),
        key=lambda t: len(t[1]) % SLOTS,
    )
    lo, hi = 0, len(rn) - 1
    while lo < hi:
        if (len(rn[lo][1]) % SLOTS) + (len(rn[hi][1]) % SLOTS) <= SLOTS:
            units.append([rn[lo], rn[hi]])
            lo += 1
            hi -= 1
        else:
            units.append([rn[hi]])
            hi -= 1
    if lo == hi:
        units.append([rn[lo]])

    bins = []
    for unit in units:
        stream = [(z, e) for z, es in unit for e in es]
        for s0 in range(0, len(stream), SLOTS):
            chunk = stream[s0 : s0 + SLOTS]
            zs = []
            for z, _ in chunk:
                if z not in zs:
                    zs.append(z)
            assert len(zs) <= 2
            zA = zs[0]
            zB = zs[1] if len(zs) > 1 else zs[0]
            binE = [e for _, e in chunk]
            half = [0 if z == zA else 1 for z, _ in chunk]
            bins.append((binE, half, zA, zB))
    assert len(bins) <= NG, f"packing produced {len(bins)} bins > {NG}"
    while len(bins) < NG:
        bins.append(([], [], 0, 0))
    return bins


def _host_prep(xyz, n_photons, coeffs, inv_voxel_size, psf_center):
    u = xyz * inv_voxel_size  # (B,N,3)
    u = u.copy()
    u[..., :2] -= psf_center[:2]
    u[..., 2] += psf_center[2]
    u_floor = np.floor(u)
    frac = u - u_floor
    ui = u_floor.astype(np.int32)
    x_idx = ui[..., 0] + PW  # (B,N)
    y_idx = ui[..., 1] + PH
    z_idx = ui[..., 2]
    frac[..., :2] = 1.0 - frac[..., :2]

    # 64-term series: series[b,n,c], c = kz*16 + kx*4 + ky
    p = frac[..., None] ** np.arange(4, dtype=np.float32)  # (B,N,3,4)
    vx, vy, vz = p[..., 0, :], p[..., 1, :], p[..., 2, :]
    series = (
        vz[..., :, None, None] * vx[..., None, :, None] * vy[..., None, None, :]
    ).reshape(B, N, C)
    s16 = n_photons[..., None] * series[:, :, None, :]  # (B,N,F,C)

    coeffs_t = np.ascontiguousarray(
        coeffs.reshape(D, HW, C).transpose(0, 2, 1)
    ).astype(ml_dtypes.bfloat16)  # (D, C, HW)

    off_all = ((y_idx - ROW0) * CSTRIDE + (x_idx - ROW0)).astype(np.int32)

    grp = np.zeros((B, NG, 128, 128 + HW), dtype=ml_dtypes.bfloat16)
    lhsT = np.zeros((128, 128), dtype=np.float32)
    offs_v = np.zeros((B, 1, N_DVE + 1), dtype=np.int32)
    offs_g = np.zeros((B, 1, N_GP + 1), dtype=np.int32)

    for b in range(B):
        bins = _pack_bins(z_idx[b])
        nv = ng = 0
        for g, (binE, half, zA, zB) in enumerate(bins):
            grp[b, g, 0:64, 128:] = coeffs_t[zA]
            grp[b, g, 64:128, 128:] = coeffs_t[zB]
            lhsT[:] = 0.0
            for i in range(SLOTS):
                if i < len(binE):
                    e, h = binE[i], half[i]
                    lhsT[h * 64 : h * 64 + 64, i * SW : i * SW + F] = s16[b, e].T
                    off = off_all[b, e]
                else:
                    off = 0  # dead slot: zero weights, scatter adds zeros
                if _gp_slot(g, i):
                    offs_g[b, 0, ng] = off
                    ng += 1
                else:
                    offs_v[b, 0, nv] = off
                    nv += 1
            grp[b, g, :, 0:128] = lhsT
        assert nv == N_DVE and ng == N_GP

    return grp, offs_v, offs_g


def make_in_maps(np_inputs):
    grp, offs_v, offs_g = _host_prep(
        np.asarray(np_inputs["xyz"], dtype=np.float32),
        np.asarray(np_inputs["n_photons"], dtype=np.float32),
        np.asarray(np_inputs["coeffs"], dtype=np.float32),
        np.asarray(np_inputs["inv_voxel_size"], dtype=np.float32),
        np.asarray(np_inputs["psf_center"], dtype=np.float32),
    )
    return [
        {
            "grp": grp[b],
            "offs_v": offs_v[b],
            "offs_g": offs_g[b],
        }
        for b in range(B)
    ]


def get_compiled():
    global _compiled
    if _compiled is None:
        _compiled = _build_bass()
    return _compiled


def kernel(xyz, n_photons, coeffs, inv_voxel_size, psf_center, img_size):
    in_maps = make_in_maps(
        {
            "xyz": xyz,
            "n_photons": n_photons,
            "coeffs": coeffs,
            "inv_voxel_size": inv_voxel_size,
            "psf_center": psf_center,
        }
    )
    nc = get_compiled()
    res = run_bass_kernel_spmd(nc, in_maps, core_ids=list(range(B)))
    out = np.stack(
        [
            res.results[b]["out"]
            .astype(np.float32)
            .reshape(3, F, IMG, IMG)
            .sum(axis=0)
            for b in range(B)
        ],
        axis=0,
    )
    return out


# revision 27
# speedup vs baseline: 1.5110x; 1.2455x over previous
import sys

sys.path.insert(0, "/opt/trn_rl_repo")

import numpy as np
import ml_dtypes

import concourse.bass as bass
import concourse.tile as tile
from concourse import bacc, mybir
from concourse.bass_utils import run_bass_kernel_spmd

# Problem constants (hardcoded per contract)
B, N, F = 8, 512, 16
D, PH, PW = 150, 26, 26
IMG = 128
C = 64  # spline coefficients per voxel
HW = PH * PW  # 676
NG = 132  # groups of 4 emitters; each group contracts K=128 = 2 stacked slabs
SLOTS = 4  # emitters per group; each slot owns 32 partitions (16 real + pad)
SW = 32  # slot width in partitions (engine partition bases must be 32-aligned)
ROW0 = 14  # min y_idx / x_idx given input coordinate ranges
CSTRIDE = 152  # canvas row stride (rows/cols span [14, 165) -> 151 + spill)
CV_ELEMS = 152 * CSTRIDE  # canvas elems per partition (incl. ds-claim spill)
OFF_MAX = (139 - ROW0) * CSTRIDE + (139 - ROW0)  # largest scatter offset

# Engine map: slot 0 on GPSIMD (GPSIMD ops must have ALL operands at
# partition base 0 -- other bases hard-crash the runtime).  Slots 1-3 on
# DVE, whose datapath handles in1 at partition bases 32/64/96 with out at
# base 0 (dynamic-offset ops bypass the same-base verifier rule and work).
USE_GP_SCATTER = True


def _gp_slot(g, i):
    if not USE_GP_SCATTER:
        return False
    return i == 0


N_GP = sum(_gp_slot(g, i) for g in range(NG) for i in range(SLOTS))
N_DVE = NG * SLOTS - N_GP
_compiled = None


def _build_bass():
    nc = bacc.Bacc()
    f32 = mybir.dt.float32
    bf16 = mybir.dt.bfloat16
    i32 = mybir.dt.int32

    # packed per-group buffer: [:, 0:128] lhsT, [:, 128:804] rhs slabs
    grp_d = nc.declare_dram_parameter("grp", [NG, 128, 128 + HW], bf16, isOutput=False)
    offs_v_d = nc.declare_dram_parameter("offs_v", [1, N_DVE + 1], i32, isOutput=False)
    offs_g_d = nc.declare_dram_parameter("offs_g", [1, N_GP + 1], i32, isOutput=False)
    # per-engine canvas crops DMA'd out separately; host sums them
    out_d = nc.declare_dram_parameter("out", [2, F, IMG * IMG], bf16, isOutput=True)

    with tile.TileContext(nc) as tc:
        with (
            tc.tile_pool(name="canvas", bufs=1) as canvas_pool,
            tc.tile_pool(name="weights", bufs=4) as w_pool,
            tc.tile_pool(name="slabs", bufs=4) as s_pool,
            tc.tile_pool(name="psum", bufs=3, space="PSUM") as p_pool,
            tc.tile_pool(name="small", bufs=1) as small_pool,
        ):
            # two canvas tiles, both at partition base 0
            canvas_v = canvas_pool.tile([SW, CV_ELEMS], bf16, tag="cv")
            canvas_g = canvas_pool.tile([SW, CV_ELEMS], bf16, tag="cg")
            nc.scalar.memzero(canvas_v[:])
            nc.scalar.memzero(canvas_g[:])

            offs_v_t = small_pool.tile([1, N_DVE + 1], i32)
            offs_g_t = small_pool.tile([1, N_GP + 1], i32)
            nc.sync.dma_start(offs_v_t[:], offs_v_d[:])
            nc.sync.dma_start(offs_g_t[:], offs_g_d[:])
            preg_v = nc.vector.alloc_register64("offp_v")
            preg_g = nc.gpsimd.alloc_register64("offp_g")

            nv = ng = 0
            for g in range(NG):
                gt = s_pool.tile([128, 128 + HW], bf16, tag="gt")
                nc.sync.dma_start(gt[:], grp_d[g])
                lt = gt[:, 0:128]
                rt = gt[:, 128 : 128 + HW]
                ps = p_pool.tile([128, HW], f32, tag="ps")
                for n0, n1 in ((0, 512), (512, HW)):
                    nc.tensor.matmul(
                        ps[:, n0:n1],
                        lhsT=lt,
                        rhs=rt[:, n0:n1],
                        start=True,
                        stop=True,
                    )
                sbp = s_pool.tile([128, HW], bf16, tag="sbp")
                nc.scalar.copy(out=sbp[:], in_=ps[:])
                ps3 = sbp[:].rearrange("p (h w) -> p h w", h=PH, w=PW)
                for i in range(SLOTS):
                    if _gp_slot(g, i):
                        eng, preg, offs_t, k = nc.gpsimd, preg_g, offs_g_t, ng
                        cnv = canvas_g  # slot 0: all operands at base 0
                        ng += 1
                    else:
                        eng, preg, offs_t, k = nc.vector, preg_v, offs_v_t, nv
                        cnv = canvas_v
                        nv += 1
                    if k % 2 == 0:
                        eng.reg_load(preg, offs_t[0:1, k : k + 2])
                    off = eng.snap(
                        preg.lo if k % 2 == 0 else preg.hi,
                        donate=True,
                        min_val=0,
                        max_val=OFF_MAX,
                    )
                    dst = (
                        cnv[:, bass.ds(off, PH * CSTRIDE)]
                        .rearrange("p (h w) -> p h w", h=PH)[:, :, 0:PW]
                    )
                    eng.tensor_tensor(
                        out=dst,
                        in0=dst,
                        in1=ps3[SW * i : SW * (i + 1)],
                        op=mybir.AluOpType.add,
                    )

            # DMA each engine's canvas crop out; host sums the two.
            # crop: rows/cols [26, 154) -> canvas-local [12, 140)
            crop_off = 12 * CSTRIDE + 12
            s1 = crop_off + IMG * CSTRIDE
            for j, cnv in enumerate((canvas_v, canvas_g)):
                cb = cnv[0:F, crop_off:s1].rearrange(
                    "p (h w) -> p h w", w=CSTRIDE
                )[:, :, 0:IMG]
                nc.sync.dma_start(
                    out_d[j].rearrange("p (h w) -> p h w", h=IMG, w=IMG),
                    cb,
                )
    if not nc.is_finalized():
        nc.finalize()
    return nc


def _pack_bins(z_idx):
    """Pack 512 emitters into bins of SLOTS slots, each bin drawing from at
    most 2 distinct z-buckets.  Buckets are grouped into units (singletons or
    pairs whose remainders mod SLOTS sum to <= SLOTS); each unit is laid out
    sequentially so every bin touches at most 2 buckets.  Returns list of NG
    bins: (emitter_idx_list, half_list, zA, zB); short bins = dead slots."""
    import collections

    buckets = collections.defaultdict(list)
    for e, z in enumerate(z_idx):
        buckets[int(z)].append(e)
    items = list(buckets.items())
    units = [[it] for it in items if len(it[1]) % SLOTS == 0]
    rn = sorted(
        (it for it in items if len(it[1]) % SLOTS != 0),
        key=lambda t: len(t[1]) % SLOTS,
    )
    lo, hi = 0, len(rn) - 1
    while lo < hi:
        if (len(rn[lo][1]) % SLOTS) + (len(rn[hi][1]) % SLOTS) <= SLOTS:
            units.append([rn[lo], rn[hi]])
            lo += 1
            hi -= 1
        else:
            units.append([rn[hi]])
            hi -= 1
    if lo == hi:
        units.append([rn[lo]])

    bins = []
    for unit in units:
        stream = [(z, e) for z, es in unit for e in es]
        for s0 in range(0, len(stream), SLOTS):
            chunk = stream[s0 : s0 + SLOTS]
            zs = []
            for z, _ in chunk:
                if z not in zs:
                    zs.append(z)
            assert len(zs) <= 2
            zA = zs[0]
            zB = zs[1] if len(zs) > 1 else zs[0]
            binE = [e for _, e in chunk]
            half = [0 if z == zA else 1 for z, _ in chunk]
            bins.append((binE, half, zA, zB))
    assert len(bins) <= NG, f"packing produced {len(bins)} bins > {NG}"
    while len(bins) < NG:
        bins.append(([], [], 0, 0))
    return bins


def _host_prep(xyz, n_photons, coeffs, inv_voxel_size, psf_center):
    u = xyz * inv_voxel_size  # (B,N,3)
    u = u.copy()
    u[..., :2] -= psf_center[:2]
    u[..., 2] += psf_center[2]
    u_floor = np.floor(u)
    frac = u - u_floor
    ui = u_floor.astype(np.int32)
    x_idx = ui[..., 0] + PW  # (B,N)
    y_idx = ui[..., 1] + PH
    z_idx = ui[..., 2]
    frac[..., :2] = 1.0 - frac[..., :2]

    # 64-term series: series[b,n,c], c = kz*16 + kx*4 + ky
    p = frac[..., None] ** np.arange(4, dtype=np.float32)  # (B,N,3,4)
    vx, vy, vz = p[..., 0, :], p[..., 1, :], p[..., 2, :]
    series = (
        vz[..., :, None, None] * vx[..., None, :, None] * vy[..., None, None, :]
    ).reshape(B, N, C)
    s16 = n_photons[..., None] * series[:, :, None, :]  # (B,N,F,C)

    coeffs_t = np.ascontiguousarray(
        coeffs.reshape(D, HW, C).transpose(0, 2, 1)
    ).astype(ml_dtypes.bfloat16)  # (D, C, HW)

    off_all = ((y_idx - ROW0) * CSTRIDE + (x_idx - ROW0)).astype(np.int32)

    grp = np.zeros((B, NG, 128, 128 + HW), dtype=ml_dtypes.bfloat16)
    lhsT = np.zeros((128, 128), dtype=np.float32)
    offs_v = np.zeros((B, 1, N_DVE + 1), dtype=np.int32)
    offs_g = np.zeros((B, 1, N_GP + 1), dtype=np.int32)

    for b in range(B):
        bins = _pack_bins(z_idx[b])
        nv = ng = 0
        for g, (binE, half, zA, zB) in enumerate(bins):
            grp[b, g, 0:64, 128:] = coeffs_t[zA]
            grp[b, g, 64:128, 128:] = coeffs_t[zB]
            # steer an odd-x emitter into slot 0 (GPSIMD); DVE keeps even-x
            # ops 4B-aligned for the bf16 2x DVE mode.  Swap only within the
            # zA block (slot half assignment must be preserved).
            if USE_GP_SCATTER and len(binE) > 1:
                for j in range(len(binE)):
                    if half[j] != 0:
                        break
                    if x_idx[b, binE[j]] % 2 == 1:
                        binE[0], binE[j] = binE[j], binE[0]
                        break
            lhsT[:] = 0.0
            for i in range(SLOTS):
                if i < len(binE):
                    e, h = binE[i], half[i]
                    lhsT[h * 64 : h * 64 + 64, i * SW : i * SW + F] = s16[b, e].T
                    off = off_all[b, e]
                else:
                    off = 0  # dead slot: zero weights, scatter adds zeros
                if _gp_slot(g, i):
                    offs_g[b, 0, ng] = off
                    ng += 1
                else:
                    offs_v[b, 0, nv] = off
                    nv += 1
            grp[b, g, :, 0:128] = lhsT
        assert nv == N_DVE and ng == N_GP

    return grp, offs_v, offs_g


def make_in_maps(np_inputs):
    grp, offs_v, offs_g = _host_prep(
        np.asarray(np_inputs["xyz"], dtype=np.float32),
        np.asarray(np_inputs["n_photons"], dtype=np.float32),
        np.asarray(np_inputs["coeffs"], dtype=np.float32),
        np.asarray(np_inputs["inv_voxel_size"], dtype=np.float32),
        np.asarray(np_inputs["psf_center"], dtype=np.float32),
    )
    return [
        {
            "grp": grp[b],
            "offs_v": offs_v[b],
            "offs_g": offs_g[b],
        }
        for b in range(B)
    ]


def get_compiled():
    global _compiled
    if _compiled is None:
        _compiled = _build_bass()
    return _compiled


def kernel(xyz, n_photons, coeffs, inv_voxel_size, psf_center, img_size):
    in_maps = make_in_maps(
        {
            "xyz": xyz,
            "n_photons": n_photons,
            "coeffs": coeffs,
            "inv_voxel_size": inv_voxel_size,
            "psf_center": psf_center,
        }
    )
    nc = get_compiled()
    res = run_bass_kernel_spmd(nc, in_maps, core_ids=list(range(B)))
    out = np.stack(
        [
            res.results[b]["out"]
            .astype(np.float32)
            .reshape(2, F, IMG, IMG)
            .sum(axis=0)
            for b in range(B)
        ],
        axis=0,
    )
    return out
